# revision 1
# baseline (speedup 1.0000x reference)
"""KAN layer (cubic B-spline, uniform grid) for 8 Trainium2 NeuronCores.

Math: with u = 1.5*x + 4.5 clamped to [0, 9], the 6 cubic B-spline basis
functions are basis_j(x) = N(u - j), where N is the uniform cubic B-spline
bump on [0, 4].  Using the finite-difference identity
    N(v) = (1/6) * sum_m a_m * relu(v - m)^3,   a = (1, -4, 6, -4, 1),
each basis function is a fixed linear combination of one-sided cubes
relu(u-k)^3 (left-anchored) or relu(q-u)^3 (mirror-anchored).  Basis j in
{3,4,5} uses left planes L_k = relu(u-k)^3, k = 3..8; basis j in {0,1,2}
uses mirror planes M_q = relu(q-u)^3, q = 1..6 (each one-sided family
vanishes identically outside its window, so plane values stay <= 216 and
fp16 storage is accurate).  The per-basis combination coefficients are
folded into the weights on the host:
    spline[n,o] = sum_{d,p} plane_p[n,d] * V[o,d,p]          (K = 512*12)
    y = spline + silu(x) @ scale_base^T + bias               (K = 512)
Planes are computed on-chip as r * s with r = relu(+-(u-k)) (vector engine,
fp16 out) and s = (u-k)^2 (scalar engine Square, fp16 out); the matmuls run
in fp16 (full PE rate, exact products, fp32 PSUM accumulation).  The mirror
planes are produced negated (min(u-q,0) * (u-q)^2 = -M_q^3); their V
columns carry the compensating sign.

Data parallel over tokens: core c processes batch row c (2048 tokens).
"""

import numpy as np

import concourse.bass as bass
import concourse.mybir as mybir
import concourse.tile as tile
from concourse import bacc
from concourse.bass_utils import run_bass_kernel_spmd

F32 = mybir.dt.float32
F16 = mybir.dt.float16
ALU = mybir.AluOpType
AF = mybir.ActivationFunctionType

N_CORES = 8
D = 512          # in_features
O = 512          # out_features
TOK = 2048       # tokens per core
NPL = 12         # spline planes: L_3..L_8 then M_1..M_6
G = 512          # tokens per plane-group
NG = TOK // G
CPG = G // 128   # matmul chunks per group
DT = D // 128    # d-tiles

# per-plane anchors: (is_mirror, anchor)
PLANES = [(False, k) for k in range(3, 9)] + [(True, q) for q in range(1, 7)]

_prog_cache = {}
last_results = None  # BassKernelResults of the most recent run (for test.py)


def _build_program(trace_names=False):
    nc = bacc.Bacc("TRN2", target_bir_lowering=False, debug=False,
                   num_devices=N_CORES)
    xT_d = nc.dram_tensor("xT", [D, TOK], F32, kind="ExternalInput").ap()
    w2_d = nc.dram_tensor("w2", [128, NPL * DT * O], F16, kind="ExternalInput").ap()
    sb_d = nc.dram_tensor("sbT", [128, DT * O], F16, kind="ExternalInput").ap()
    bc_d = nc.dram_tensor("bcols", [128, NPL], F32, kind="ExternalInput").ap()
    br_d = nc.dram_tensor("biasrep", [128, O], F32, kind="ExternalInput").ap()
    y_d = nc.dram_tensor("y", [TOK, O], F32, kind="ExternalOutput").ap()

    with tile.TileContext(nc) as tc:
        with tc.tile_pool(name="const", bufs=1) as cpool, \
             tc.tile_pool(name="xg", bufs=2) as xpool, \
             tc.tile_pool(name="work", bufs=3) as wpool, \
             tc.tile_pool(name="planes", bufs=1) as ppool, \
             tc.tile_pool(name="outp", bufs=2) as opool, \
             tc.tile_pool(name="psum", bufs=4, space="PSUM") as pspool:

            bcols = cpool.tile([128, NPL], F32, name="bcols_t", tag="bcols")
            nc.sync.dma_start(bcols[:], bc_d[:])
            biasr = cpool.tile([128, O], F32, name="biasr_t", tag="biasr")
            nc.sync.dma_start(biasr[:], br_d[:])
            w2all = cpool.tile([128, NPL * DT * O], F16, name="w2all", tag="w2all")
            nc.gpsimd.dma_start(w2all[:], w2_d[:])
            sball = cpool.tile([128, DT * O], F16, name="sball", tag="sball")
            nc.gpsimd.dma_start(sball[:], sb_d[:])
            w2t = [w2all[:, i * O:(i + 1) * O] for i in range(NPL * DT)]
            sbt = [sball[:, t_ * O:(t_ + 1) * O] for t_ in range(DT)]

            wones = cpool.tile([1, O], F16, name="wones", tag="wones")
            nc.vector.memset(wones[:], 0.0)
            wps = pspool.tile([128, O], F32, name="wps", tag="wps", bufs=1)
            for _w in range(24):
                nc.tensor.matmul(wps[:], wones[:, 0:128], wones[:],
                                 start=True, stop=True)

            for g in range(NG):
                xg = xpool.tile([128, DT * G], F32, name="xg", tag="xg")
                for t_ in range(DT):
                    nc.sync.dma_start(
                        xg[:, t_ * G:(t_ + 1) * G],
                        xT_d[t_ * 128:(t_ + 1) * 128, g * G:(g + 1) * G])
                u = wpool.tile([128, DT * G], F32, name="u", tag="u", bufs=2)
                nc.vector.tensor_scalar(u[:], xg[:], 1.5, 4.5, ALU.mult, ALU.add)
                nc.vector.tensor_scalar(u[:], u[:], 9.0, 0.0, ALU.min, ALU.max)

                planes = []
                for p, (mirror, k) in enumerate(PLANES):
                    r = wpool.tile([128, DT * G], F16, name=f"r{p}", tag="r", bufs=2)
                    if mirror:
                        nc.vector.tensor_scalar(r[:], u[:], -float(k), 0.0,
                                                ALU.add, ALU.min)
                    else:
                        nc.vector.tensor_scalar(r[:], u[:], -float(k), 0.0,
                                                ALU.add, ALU.max)
                    q = wpool.tile([128, DT * G], F16, name=f"q{p}", tag="q", bufs=2)
                    nc.scalar.activation(q[:], u[:], AF.Square,
                                         bias=bcols[:, p:p + 1], scale=1.0)
                    pl = ppool.tile([128, DT * G], F16, name=f"pl{p}",
                                    tag=f"pl{p}", bufs=2)
                    nc.vector.tensor_mul(pl[:], r[:], q[:])
                    planes.append(pl)
                sil = ppool.tile([128, DT * G], F16, name="sil", tag="sil", bufs=1)
                nc.scalar.activation(sil[:], xg[:], AF.Silu)

                for c in range(CPG):
                    ps = pspool.tile([128, O], F32, name="ps", tag="ps")
                    n_mm = NPL * DT + DT
                    i = 0
                    for p in range(NPL):
                        for t_ in range(DT):
                            sl = planes[p][:, t_ * G + c * 128:
                                           t_ * G + (c + 1) * 128]
                            nc.tensor.matmul(ps[:], sl, w2t[p * DT + t_],
                                             start=(i == 0),
                                             stop=(i == n_mm - 1))
                            i += 1
                    for t_ in range(DT):
                        sl = sil[:, t_ * G + c * 128:t_ * G + (c + 1) * 128]
                        nc.tensor.matmul(ps[:], sl, sbt[t_],
                                         start=False, stop=(i == n_mm - 1))
                        i += 1
                    ot = opool.tile([128, O], F32, name="ot", tag="ot")
                    nc.vector.scalar_tensor_tensor(ot[:], ps[:], 1.0, biasr[:],
                                                   ALU.mult, ALU.add)
                    nc.scalar.dma_start(
                        y_d[g * G + c * 128:g * G + (c + 1) * 128, :], ot[:])
    nc.compile()
    return nc


def _host_tables(coef, scale_base, scale_sp, bias):
    W = (scale_sp[..., None] * coef).astype(np.float64)        # (O, D, 6)
    a = np.array([1., -4., 6., -4., 1.]) / 6.0
    V = np.zeros((O, D, NPL))
    for j in (3, 4, 5):                    # left-anchored
        for m in range(5):
            k = j + m
            if k <= 8:
                V[:, :, k - 3] += a[m] * W[:, :, j]
    for j in (0, 1, 2):                    # mirror-anchored (planes negated)
        for m in range(5):
            q = j + 4 - m
            if q >= 1:
                V[:, :, 6 + q - 1] -= a[m] * W[:, :, j]
    w2s = np.empty((NPL * DT, 128, O), np.float16)
    for p in range(NPL):
        for t in range(DT):
            w2s[p * DT + t] = V[:, t * 128:(t + 1) * 128, p].T
    w2 = np.ascontiguousarray(w2s.transpose(1, 0, 2).reshape(128, NPL * DT * O))
    sbs = np.empty((DT, 128, O), np.float16)
    for t in range(DT):
        sbs[t] = scale_base[:, t * 128:(t + 1) * 128].T
    sbT = np.ascontiguousarray(sbs.transpose(1, 0, 2).reshape(128, DT * O))
    bcols = np.tile(-np.array([k for (_, k) in PLANES], np.float32), (128, 1))
    biasrep = np.tile(bias.astype(np.float32), (128, 1))
    return w2, sbT, np.ascontiguousarray(bcols), np.ascontiguousarray(biasrep)


def kernel(x, coef, scale_base, scale_sp, bias, _trace=False):
    global last_results
    x = np.asarray(x, np.float32)
    coef = np.asarray(coef, np.float32)
    scale_base = np.asarray(scale_base, np.float32)
    scale_sp = np.asarray(scale_sp, np.float32)
    bias = np.asarray(bias, np.float32)
    B, S, Din = x.shape
    assert (B * S, Din) == (N_CORES * TOK, D), (x.shape,)

    if "nc" not in _prog_cache:
        _prog_cache["nc"] = _build_program()
    nc = _prog_cache["nc"]

    w2, sbT, bcols, biasrep = _host_tables(coef, scale_base, scale_sp, bias)
    xflat = x.reshape(N_CORES, TOK, D)
    in_maps = []
    for c in range(N_CORES):
        in_maps.append({
            "xT": np.ascontiguousarray(xflat[c].T),
            "w2": w2, "sbT": sbT, "bcols": bcols, "biasrep": biasrep,
        })
    kw = {}
    if _trace:
        kw.update(trace=True)
    last_results = run_bass_kernel_spmd(nc, in_maps,
                                        core_ids=list(range(N_CORES)), **kw)
    y = np.stack([last_results.results[c]["y"] for c in range(N_CORES)], 0)
    return y.reshape(B, S, O).astype(np.float32)



# revision 4
# speedup vs baseline: 1.5405x; 1.5405x over previous
"""KAN layer (cubic B-spline, uniform grid) for 8 Trainium2 NeuronCores.

Math: with u = 1.5*x + 4.5, basis_j(x) = N(u - j) where N is the uniform
cubic B-spline bump on [0, 4].  N decomposes into two tent-cubes:
    N(v) = (A^3 - 4*B^3) / 6,  A = relu(min(v, 4-v)),  B = relu(min(v-1, 3-v))
(A is the height-2 tent over [0,4], B the height-1 tent over [1,3]; both
vanish outside, so basis values are exactly zero out of support and plane
magnitudes stay <= 8 -- safe to quantize to fp8e4 with no cancellation.)

Two custom DVE ops evaluate this per element from t = 1.5*x (computed once
on the scalar engine):  CUBE_A produces A^3 = relu(min(t+s0, s1-t))^3 in
fp16; CUBE_BC produces the combined basis feature A^3 - 4*B^3 in fp8.
Six fp8 basis features + fp16 silu feed the matmuls; spline weights are
quantized to fp8e4 (scaled x64 to avoid subnormals) and run as DoubleRow
fp8 matmuls (2 contraction k-tiles per instruction, 2x PE throughput).
The base path (silu @ scale_base^T, x64 in fp16) accumulates into the
same PSUM; the drain applies x(1/64) and adds the bias.

Data parallel over tokens: core c processes batch row c (2048 tokens).
"""

import numpy as np
import ml_dtypes

import concourse.bass as bass
import concourse.mybir as mybir
import concourse.tile as tile
from concourse import bacc
from concourse.bass_utils import run_bass_kernel_spmd

F32 = mybir.dt.float32
F16 = mybir.dt.float16
F8 = mybir.dt.float8e4
ALU = mybir.AluOpType
AF = mybir.ActivationFunctionType
DRM = mybir.MatmulPerfMode.DoubleRow
NP8 = ml_dtypes.float8_e4m3

N_CORES = 8
D = 512          # in_features
O = 512          # out_features
TOK = 2048       # tokens per core
NJ = 6           # spline basis functions
G = 1024         # tokens per group
NG = TOK // G
CPG = G // 128   # output chunks per group
DT = D // 128    # d-tiles
NPR = NJ * 2     # DoubleRow pairs per chunk (2 d-pairs per basis)
SC = 64.0        # weight scale (fp8 subnormal avoidance)

_prog_cache = {}
last_results = None  # BassKernelResults of the most recent run (for test.py)


def _register_ops():
    """Register the two custom DVE ops (idempotent)."""
    import concourse.dve_ops as dve_ops
    from concourse.dve_ops import DveOp
    from concourse.dve_spec import (Spec, Src0, Src1, C0, C1, C2, relu, sq,
                                    minn, lower, _has_src1)
    from concourse.dve_uop import DveOpSpec

    def reg(name, spec):
        for op in dve_ops.OPS:
            if op.name == name:
                return op
        opcode = dve_ops._CUSTOM_DVE_ROW_BASE + len(dve_ops.OPS)
        shas = {}
        for ver in ("v3", "v4"):
            s = DveOpSpec(name=name, opcode=opcode, uops=lower(spec, ver=ver),
                          rd1_en=_has_src1(spec))
            shas[ver] = s.sha(ver)
        op = DveOp(name, spec, subdim=False, uops_sha=shas)
        dve_ops.OPS.append(op)
        dve_ops._SUB_OPCODE_FOR_NAME[name] = opcode
        dve_ops.CUSTOM_DVE_SPECS[name] = spec
        return op

    p = Src0 + C0
    q = C1 - Src0
    m = minn(p, q)
    spec_a = Spec(
        body=sq(m) * relu(m),
        reference=lambda in0, in1, s0, s1, imm2: np.maximum(
            np.minimum(in0.astype(np.float32) + s0,
                       s1 - in0.astype(np.float32)), 0.0) ** 3)
    p2 = Src0 + C0
    q2 = C1 - Src0
    m2 = minn(p2, q2)
    spec_b = Spec(
        body=sq(m2) * relu(m2) * C2 + Src1,
        reference=lambda in0, in1, s0, s1, imm2: imm2 * np.maximum(
            np.minimum(in0.astype(np.float32) + s0,
                       s1 - in0.astype(np.float32)), 0.0) ** 3
            + in1.astype(np.float32))
    return reg("KAN_CUBE_A_ANT", spec_a), reg("KAN_CUBE_BC_ANT", spec_b)


def _build_program():
    cube_a, cube_bc = _register_ops()
    nc = bacc.Bacc("TRN2", target_bir_lowering=False, debug=False,
                   num_devices=N_CORES)
    xT_d = nc.dram_tensor("xT", [D, TOK], F32, kind="ExternalInput").ap()
    w8_d = nc.dram_tensor("w8", [128, NPR, 2, O], F8, kind="ExternalInput").ap()
    sb_d = nc.dram_tensor("sbT", [128, DT * O], F16, kind="ExternalInput").ap()
    br_d = nc.dram_tensor("biasrow", [1, O], F16, kind="ExternalInput").ap()
    y_d = nc.dram_tensor("y", [TOK, O], F32, kind="ExternalOutput").ap()

    with tile.TileContext(nc) as tc:
        with tc.tile_pool(name="const", bufs=1) as cpool, \
             tc.tile_pool(name="xg", bufs=2) as xpool, \
             tc.tile_pool(name="work", bufs=2) as wpool, \
             tc.tile_pool(name="planes", bufs=2) as ppool, \
             tc.tile_pool(name="outp", bufs=2) as opool, \
             tc.tile_pool(name="psum", bufs=6, space="PSUM") as pspool:

            w8all = cpool.tile([128, NPR, 2, O], F8, name="w8all", tag="w8all")
            nc.gpsimd.dma_start(w8all[:], w8_d[:])
            sball = cpool.tile([128, DT * O], F16, name="sball", tag="sball")
            nc.gpsimd.dma_start(sball[:], sb_d[:])
            biasr = cpool.tile([1, O], F16, name="biasr", tag="biasr")
            nc.sync.dma_start(biasr[:], br_d[:])
            sbt = [sball[:, t_ * O:(t_ + 1) * O] for t_ in range(DT)]

            wones = cpool.tile([1, O], F16, name="wones", tag="wones")
            nc.vector.memset(wones[:], 1.0)
            wps = pspool.tile([128, O], F32, name="wps", tag="wps", bufs=1)
            for _w in range(24):
                nc.tensor.matmul(wps[:], wones[:, 0:128], wones[:],
                                 start=True, stop=True)

            for g in range(NG):
                xg = xpool.tile([128, DT, G], F32, name="xg", tag="xg")
                for t_ in range(DT):
                    nc.sync.dma_start(
                        xg[:, t_, :],
                        xT_d[t_ * 128:(t_ + 1) * 128, g * G:(g + 1) * G])
                xs = wpool.tile([128, DT, G], F16, name="xs", tag="xs")
                nc.scalar.activation(xs[:], xg[:], AF.Copy,
                                     bias=0.0, scale=1.5)
                sil = ppool.tile([128, DT, G], F16, name="sil", tag="sil")
                nc.scalar.activation(sil[:], xg[:], AF.Silu)

                basis = []
                for j in range(NJ):
                    a3 = wpool.tile([128, DT * G], F16, name=f"a3_{j}",
                                    tag="a3")
                    nc.vector._custom_dve(cube_a, out=a3[:], in0=xs[:],
                                          s0=4.5 - j, s1=j - 0.5)
                    bj = ppool.tile([128, DT, G], F8, name=f"b{j}",
                                    tag=f"b{j}")
                    nc.vector._custom_dve(cube_bc, out=bj[:], in0=xs[:],
                                          in1=a3[:], s0=3.5 - j, s1=j - 1.5,
                                          imm2=-4.0)
                    basis.append(bj)

                for c in range(CPG):
                    ps = pspool.tile([128, O], F32, name="ps", tag="ps")
                    n_mm = NPR + DT
                    nc.tensor.matmul(ps[:], wones[:, 0:128], biasr[:],
                                     start=True, stop=False)
                    i = 0
                    for j in range(NJ):
                        for tp in range(2):
                            lhsT = basis[j][:, 2 * tp:2 * tp + 2,
                                            c * 128:(c + 1) * 128]
                            nc.tensor.matmul(ps[:], lhsT,
                                             w8all[:, j * 2 + tp, :, :],
                                             start=False, stop=False,
                                             perf_mode=DRM)
                            i += 1
                    for t_ in range(DT):
                        sl = sil[:, t_, c * 128:(c + 1) * 128]
                        nc.tensor.matmul(ps[:], sl, sbt[t_],
                                         start=False, stop=(i == n_mm - 1))
                        i += 1
                    ot = opool.tile([128, O], F32, name="ot", tag="ot")
                    nc.scalar.activation(ot[:], ps[:], AF.Copy,
                                         bias=0.0, scale=1.0 / SC)
                    nc.scalar.dma_start(
                        y_d[g * G + c * 128:g * G + (c + 1) * 128, :], ot[:])
    nc.compile()
    return nc


def _host_tables(coef, scale_base, scale_sp, bias):
    W = (scale_sp[..., None] * coef).astype(np.float64)        # (O, D, 6)
    w8 = np.empty((128, NPR, 2, O), NP8)
    for j in range(NJ):
        Vj = (SC / 6.0) * W[:, :, j]                           # (O, D)
        for tp in range(2):
            for i in range(2):
                dt_ = 2 * tp + i
                w8[:, j * 2 + tp, i, :] = \
                    Vj[:, dt_ * 128:(dt_ + 1) * 128].T.astype(NP8)
    sbs = np.empty((DT, 128, O), np.float16)
    sb_scaled = SC * scale_base.astype(np.float64)
    for t in range(DT):
        sbs[t] = sb_scaled[:, t * 128:(t + 1) * 128].T
    sbT = np.ascontiguousarray(sbs.transpose(1, 0, 2).reshape(128, DT * O))
    biasrow = (SC * bias.astype(np.float64)).astype(np.float16).reshape(1, O)
    return np.ascontiguousarray(w8), sbT, np.ascontiguousarray(biasrow)


def kernel(x, coef, scale_base, scale_sp, bias, _trace=False):
    global last_results
    x = np.asarray(x, np.float32)
    coef = np.asarray(coef, np.float32)
    scale_base = np.asarray(scale_base, np.float32)
    scale_sp = np.asarray(scale_sp, np.float32)
    bias = np.asarray(bias, np.float32)
    B, S, Din = x.shape
    assert (B * S, Din) == (N_CORES * TOK, D), (x.shape,)

    if "nc" not in _prog_cache:
        _prog_cache["nc"] = _build_program()
    nc = _prog_cache["nc"]

    w8, sbT, biasrow = _host_tables(coef, scale_base, scale_sp, bias)
    xflat = x.reshape(N_CORES, TOK, D)
    in_maps = []
    for c in range(N_CORES):
        in_maps.append({
            "xT": np.ascontiguousarray(xflat[c].T),
            "w8": w8, "sbT": sbT, "biasrow": biasrow,
        })
    kw = {}
    if _trace:
        kw.update(trace=True)
    last_results = run_bass_kernel_spmd(nc, in_maps,
                                        core_ids=list(range(N_CORES)), **kw)
    y = np.stack([last_results.results[c]["y"] for c in range(N_CORES)], 0)
    return y.reshape(B, S, O).astype(np.float32)
